# revision 2
# baseline (speedup 1.0000x reference)
"""Trainium2 Bass kernel for a GPT-style causal attention block.

  y = proj( softmax_causal( (x@Wq)(x@Wk)^T / sqrt(hd) ) @ (x@Wv) )

Shapes (hardcoded): B=2, S=2048, D=1024, H=16 heads, hd=64.

Sharding over 8 NeuronCores: core = (batch b, head-group g), g selects 4
heads. Row-parallel output projection: each core computes a PARTIAL
y[b] = a_g @ Wp[rows of g's heads, :] for the full D=1024 columns, and
the host unshard sums the 4 partials per batch (standard Megatron
row-parallel sharding). No on-device collective -> no NEFF-start CC
barrier (~46us) and no AllGather (~60us) on the critical path.

Per core:
  phase 1: QKV projection for its 4 heads (fp32r matmuls).
           q,k produced TRANSPOSED  [head_ch, S]  (contraction-ready),
           v produced natural       [S, head_ch] (+ a ones column).
  phase 2: causal attention, head PAIRS processed together in the
           transposed-score layout [key, query]: the two K=64 score
           matmuls of a pair run CONCURRENTLY in the PE array via
           row-group tile_position (0,0)/(64,0); exp on ACT (scale=1/8
           folded in); causal mask multiplies only the 4 diagonal key
           tiles (emitted first so they overlap the non-diagonal
           stream; split between DVE and gpsimd); AV matmul with
           lhsT=[v|1] so psum row 64 accumulates the softmax
           denominator; normalization via ACT reciprocal (table-based,
           ~8x faster than the iterative DVE reciprocal) + a
           PE-replicated broadcast.
  phase 3 (interleaved): after pair-1's attention block ib completes,
           project seq tiles 4*ib..4*ib+4: psum[128,1024] accumulates
           lhsT=aT(pair0)+aT(pair1) against the local 256 rows of
           w_proj; only the last chunk's ~10us is exposed at the tail.

Matmuls run in float32r (full PE rate when the moving free dim is
>=256). aT and w_proj stay f32r end-to-end (no bf16 rounding; the
AllGather payload cast is gone), so accuracy improves over the
collective variant. All host-side sharding/layout prep is data-only so
the single SPMD program is rank-independent.
"""

import numpy as np

B = 2
S = 2048
D = 1024
H = 16
HD = 64
HLOC = 4          # heads per core
NPAIR = 2         # head pairs per core
N_CORES = 8
GROUP = 4         # cores per batch
IB = 512          # query block width (matmul moving dim)
JT = 128          # key tile (psum partition dim)
SCALE = 1.0 / 8.0  # 1/sqrt(hd)


def _build_bass(s=S, repeat=1, phases="all"):
    """Build the SPMD Bass program (one NeuronCore's view)."""
    import concourse.bacc as bacc
    import concourse.mybir as mybir
    import concourse.tile as tile

    f32 = mybir.dt.float32
    f32r = mybir.dt.float32r
    Alu = mybir.AluOpType
    Act = mybir.ActivationFunctionType

    n_ib = s // IB           # query blocks
    n_st = s // 128          # 128-row sequence tiles
    n_dt = D // 128          # contraction tiles for D

    nc = bacc.Bacc(num_devices=N_CORES)

    # all big inputs host-pre-arranged to SBUF layout (partition-major,
    # >=2KB contiguous per partition per DMA)
    xt = nc.declare_dram_parameter("xt", [s // IB, 128, D // 128, IB], f32r, isOutput=False)
    wqks = [
        nc.declare_dram_parameter(f"wqk{t}", [128, D // 128, 128], f32r, isOutput=False)
        for t in range(4)
    ]
    # v weights with a zero column per head (65th); + bias ones column
    # makes v_sb's denominator-ones column without any DMA
    wv = nc.declare_dram_parameter("wv", [128, D // 128, 260], f32r, isOutput=False)
    bqk = nc.declare_dram_parameter("bqk", [128, 4], f32, isOutput=False)
    bv = nc.declare_dram_parameter("bv", [128, 260], f32, isOutput=False)
    # row-parallel projection: local 256 rows of w_proj, all D columns,
    # as [128, pair, D] contraction tiles
    wp = nc.declare_dram_parameter("wp", [128, NPAIR, D], f32r, isOutput=False)
    bp = nc.declare_dram_parameter("bp", [128, D], f32, isOutput=False)
    msk = nc.declare_dram_parameter("msk", [128, 4, IB], f32, isOutput=False)
    y = nc.declare_dram_parameter("y", [s, D], f32, isOutput=True)

    with tile.TileContext(nc) as tc:
        with (
            tc.tile_pool(name="const", bufs=1) as const,
            tc.tile_pool(name="persist", bufs=1) as persist,
            tc.tile_pool(name="p1in", bufs=1) as p1in,
        ):
            # ---- input DMA, first-needed-first ----
            # qkT(pair0) needs wqk + xt slab; v_for(0..4)+attention ib0
            # needs wv, msk, xt slab0. Everything else can land later.
            # two HWDGE queues (SP + ACT), loads issued in consumption
            # order: wqk col-group t=0 (q pair0), xt slab 0, t=2 (k
            # pair0), wv, remaining slabs/cols, then phase-3 weights.
            # Every transfer is contiguous per partition (>=2KB lines).
            wqk_sbs = [
                p1in.tile([128, n_dt, 128], f32r, name=f"wqk_sb{t}")
                for t in range(4)
            ]
            xt_sb = p1in.tile([128, n_dt, s], f32r)
            wv_sb = p1in.tile([128, n_dt, 260], f32r)

            nc.scalar.dma_start(out=wqk_sbs[0], in_=wqks[0][:, :, :])
            for sh in range(n_ib):
                # split each slab across the two queues (dt halves)
                nc.sync.dma_start(
                    out=xt_sb[:, 0 : n_dt // 2, sh * IB : (sh + 1) * IB],
                    in_=xt[sh, :, 0 : n_dt // 2, :],
                )
                nc.scalar.dma_start(
                    out=xt_sb[:, n_dt // 2 :, sh * IB : (sh + 1) * IB],
                    in_=xt[sh, :, n_dt // 2 :, :],
                )
                if sh == 0:
                    nc.scalar.dma_start(out=wqk_sbs[2], in_=wqks[2][:, :, :])
                    nc.scalar.dma_start(out=wv_sb, in_=wv[:, :, :])
                    bqk_sb = const.tile([128, 4], f32)
                    nc.sync.dma_start(out=bqk_sb, in_=bqk[:, :])
                    bv_sb = const.tile([128, 260], f32)
                    nc.sync.dma_start(out=bv_sb, in_=bv[:, :])
                if sh == 1:
                    msk_sb = const.tile([128, 4, IB], f32)
                    nc.sync.dma_start(out=msk_sb, in_=msk[:, :, :])
                    nc.scalar.dma_start(out=wqk_sbs[1], in_=wqks[1][:, :, :])
                    nc.scalar.dma_start(out=wqk_sbs[3], in_=wqks[3][:, :, :])
            wp_sb = const.tile([128, NPAIR, D], f32r)
            nc.scalar.dma_start(out=wp_sb, in_=wp[:, :, :])
            bp_sb = const.tile([128, D], f32)
            nc.scalar.dma_start(out=bp_sb, in_=bp[:, :])
            # dummy exp: pulls the ACT exp table load off the critical path
            warm_sb = const.tile([1, 1], f32)
            nc.scalar.activation(
                out=warm_sb, in_=bqk_sb[0:1, 0:1], func=Act.Exp, scale=0.0
            )

            for _rep in range(repeat):
             # persistent intermediates
             qT_sb = persist.tile([128, NPAIR, s], f32r, name="qT_sb")
             kT_sb = persist.tile([128, NPAIR, s], f32r)
             v_sb = persist.tile([128, n_st, HLOC, 65], f32r)
             aT_sb = persist.tile([128, NPAIR, s], f32r)

             # v natural: lhsT = xT tile [d, s-tile], rhs = Wv [d, 256]
             def v_for(st_lo, st_hi, pool):
                 for st in range(st_lo, st_hi):
                     psv = pool.tile([128, 260], f32, name="psv", tag="pss")
                     for dt in range(n_dt):
                         nc.tensor.matmul(
                             psv,
                             lhsT=(xt_sb[:, dt, st * 128 : (st + 1) * 128]),
                             rhs=(wv_sb[:, dt, :]),
                             start=(dt == 0),
                             stop=(dt == n_dt - 1),
                         )
                     # per-head 65th col: 0 (zero weight col) + 1 (bias)
                     nc.vector.tensor_tensor(
                         out=v_sb[:, st, :, :],
                         in0=psv.rearrange("p (h e) -> p h e", h=HLOC),
                         in1=bv_sb.rearrange("p (h e) -> p h e", h=HLOC),
                         op=Alu.add,
                     )

             def qkT_one(t, sb, pool):
                 # qT/kT: lhsT = W tile [d,c], rhs = xT [d, s-block]
                 # c-tile t: 0,1 = q pair0/1; 2,3 = k pair0/1
                 ps = pool.tile([128, IB], f32, name="ps", tag="pss")
                 for dt in range(n_dt):
                     nc.tensor.matmul(
                         ps,
                         lhsT=(wqk_sbs[t][:, dt, :]),
                         rhs=(xt_sb[:, dt, sb * IB : (sb + 1) * IB]),
                         start=(dt == 0),
                         stop=(dt == n_dt - 1),
                     )
                 dst = qT_sb if t < 2 else kT_sb
                 nc.vector.tensor_scalar_add(
                     out=dst[:, t % 2, sb * IB : (sb + 1) * IB],
                     in0=ps,
                     scalar1=bqk_sb[:, t : t + 1],
                 )

             def qkT_for(t, pool):
                 for sb in range(n_ib):
                     qkT_one(t, sb, pool)

             if phases == "p1":
                 with tc.tile_pool(name="ps_p1", bufs=2, space="PSUM") as ps_p1:
                     v_for(0, n_st, ps_p1)
                     for t in range(4):
                         qkT_for(t, ps_p1)
                 continue

             # ---- attention + interleaved projection ----
             with (
                 tc.tile_pool(name="ps_s", bufs=3, space="PSUM") as ps_s,
                 tc.tile_pool(name="ps_av", bufs=1, space="PSUM") as ps_av,
                 tc.tile_pool(name="pt", bufs=3) as ptpool,
                 tc.tile_pool(name="small", bufs=2) as small,
                 tc.tile_pool(name="yout", bufs=2) as yout,
             ):
              def proj_chunk(ib):
                 # ---- projection of seq tiles 4*ib..4*ib+4 ----
                 # emitted one ib late so the next block's scores keep
                 # the PE busy while this block's normalize chain runs
                 for st in range(4 * ib, 4 * ib + 4):
                     psy = ps_s.tile([128, D], f32, name="psy", tag="pss")
                     for pp in range(NPAIR):
                         for ch in range(2):  # psum-bank halves
                             nc.tensor.matmul(
                                 psy[:, ch * 512 : (ch + 1) * 512],
                                 lhsT=(aT_sb[
                                         :, pp, st * 128 : (st + 1) * 128
                                     ]
                                 ),
                                 rhs=(wp_sb[:, pp, ch * 512 : (ch + 1) * 512]),
                                 start=(pp == 0),
                                 stop=(pp == NPAIR - 1),
                             )
                     ysb = yout.tile([128, D], f32, name="ysb")
                     nc.vector.tensor_tensor(
                         out=ysb, in0=psy, in1=bp_sb, op=Alu.add
                     )
                     yeng = nc.sync if st % 2 == 0 else nc.scalar
                     yeng.dma_start(
                         out=y[st * 128 : (st + 1) * 128, :], in_=ysb
                     )

              for pair in range(NPAIR):
                 for ib in range(n_ib):
                     # q/k col-projections for just this block: attention
                     # on block ib then runs while later slabs still DMA
                     qkT_one(pair, ib, ps_s)
                     qkT_one(2 + pair, ib, ps_s)
                     if pair == 0:
                         # v quarter-blocks on demand
                         v_for(4 * ib, 4 * ib + 4, ps_s)
                     njt = 4 * (ib + 1)  # key tiles needed (j <= i)
                     avs = [
                         ps_av.tile([65, IB], f32, name=f"av{hh}", tag=f"av{hh}")
                         for hh in range(2)
                     ]
                     # diagonal key tiles first: their mask multiply then
                     # overlaps the long non-diagonal score/AV stream
                     jt_order = list(range(4 * ib, njt)) + list(range(4 * ib))
                     for jseq, jt in enumerate(jt_order):
                         pss = ps_s.tile([128, 2 * IB], f32, name="pss")
                         for hh in range(2):
                             off = hh * 64
                             nc.tensor.matmul(
                                 pss[:, hh * IB : (hh + 1) * IB],
                                 lhsT=(kT_sb[
                                         off : off + 64,
                                         pair,
                                         jt * 128 : (jt + 1) * 128,
                                     ]
                                 ),
                                 rhs=(qT_sb[
                                         off : off + 64,
                                         pair,
                                         ib * IB : (ib + 1) * IB,
                                     ]
                                 ),
                                 start=True,
                                 stop=True,
                                 tile_position=(off, 0),
                             )
                         pt = ptpool.tile([128, 2 * IB], f32r, name="pt")
                         k = jt - 4 * ib
                         for hh in range(2):
                             # exp per psum-bank half: AV(hh) unblocks as
                             # soon as its own half is through ACT
                             nc.scalar.activation(
                                 out=pt[:, hh * IB : (hh + 1) * IB],
                                 in_=pss[:, hh * IB : (hh + 1) * IB],
                                 func=Act.Exp,
                                 scale=SCALE,
                             )
                             if k >= 0:  # diagonal tile: causal mask
                                 nc.vector.tensor_tensor(
                                     out=pt[:, hh * IB : (hh + 1) * IB],
                                     in0=pt[:, hh * IB : (hh + 1) * IB],
                                     in1=msk_sb[:, k, :],
                                     op=Alu.mult,
                                 )
                             nc.tensor.matmul(
                                 avs[hh],
                                 lhsT=(v_sb[:, jt, pair * 2 + hh, :]),
                                 rhs=(pt[:, hh * IB : (hh + 1) * IB]),
                                 start=(jseq == 0),
                                 stop=(jseq == njt - 1),
                             )
                     # normalize per head: aT = av[0:64] * (1 / av[64]).
                     # reciprocal_approx_fast on the [1,512] denominator
                     # row (~18 correct bits, plenty for softmax denoms),
                     # then a stride-0 DMA replicates it across the 64
                     # partitions -- no ACT/PE/psum involvement at all.
                     for hh in range(2):
                         av = avs[hh]
                         off = hh * 64
                         den_sb = small.tile([1, IB], f32, name="den_sb")
                         nc.scalar.activation(
                             out=den_sb,
                             in_=av[64:65, :],
                             func=Act.Copy,
                             scale=1.0,
                         )
                         rec1 = small.tile([1, IB], f32, name="rec1")
                         nc.vector.reciprocal_approx_fast(
                             out=rec1, in_=den_sb
                         )
                         rec_rep = small.tile([64, IB], f32, name="rec_rep")
                         nc.gpsimd.partition_broadcast(rec_rep, rec1)
                         nc.vector.tensor_tensor(
                             out=aT_sb[
                                 off : off + 64, pair, ib * IB : (ib + 1) * IB
                             ],
                             in0=av[0:64, :],
                             in1=rec_rep,
                             op=Alu.mult,
                         )
                     if pair == 1 and phases != "attn" and ib > 0:
                         proj_chunk(ib - 1)
                 if pair == 1 and phases != "attn":
                     proj_chunk(n_ib - 1)

    nc.compile()
    return nc


def _shard_inputs(x, w_attn, b_attn, w_proj, b_proj, s=S):
    """Host-side sharding: build the per-core input maps."""
    x = np.asarray(x, dtype=np.float32)
    w_attn = np.asarray(w_attn, dtype=np.float32)
    b_attn = np.asarray(b_attn, dtype=np.float32)
    w_proj = np.asarray(w_proj, dtype=np.float32)
    b_proj = np.asarray(b_proj, dtype=np.float32)

    # causal mask tiles: msk[j, k, i] = 1.0 if i >= j + 128*k
    jj = np.arange(128)[:, None, None]
    kk = np.arange(4)[None, :, None]
    ii = np.arange(IB)[None, None, :]
    msk = (ii >= jj + 128 * kk).astype(np.float32)

    n_dt = D // 128
    in_maps = []
    for core in range(N_CORES):
        b, g = divmod(core, GROUP)
        hs = list(range(g * HLOC, (g + 1) * HLOC))
        # xt: [sh, 128, dt, IB] so each slab is one contiguous DMA
        xtT = np.ascontiguousarray(x[b].T)  # [D, S]
        xt = np.ascontiguousarray(
            xtT.reshape(n_dt, 128, S // IB, IB).transpose(2, 1, 0, 3)
        )
        qcols = np.concatenate(
            [w_attn[:, h * HD : (h + 1) * HD] for h in hs], axis=1
        )
        kcols = np.concatenate(
            [w_attn[:, D + h * HD : D + (h + 1) * HD] for h in hs], axis=1
        )
        # wqk col-groups t=0,1 (q pairs), 2,3 (k pairs): [128, dt, 128]
        wqk_t = {}
        for t in range(4):
            blk = (qcols if t < 2 else kcols)[:, (t % 2) * 128 : (t % 2) * 128 + 128]
            wqk_t[f"wqk{t}"] = np.ascontiguousarray(
                blk.reshape(n_dt, 128, 128).transpose(1, 0, 2)
            )
        # wv with a zero 65th column per head: [D, 260] -> [128, dt, 260]
        wv_full = np.zeros((D, 260), np.float32)
        for i, h in enumerate(hs):
            wv_full[:, i * 65 : i * 65 + 64] = w_attn[
                :, 2 * D + h * HD : 2 * D + (h + 1) * HD
            ]
        wv = np.ascontiguousarray(
            wv_full.reshape(n_dt, 128, 260).transpose(1, 0, 2)
        )
        bq = np.concatenate([b_attn[h * HD : (h + 1) * HD] for h in hs])
        bk = np.concatenate([b_attn[D + h * HD : D + (h + 1) * HD] for h in hs])
        bqk = np.concatenate([bq, bk]).reshape(4, 128).T.copy()  # [128, 4]
        # v bias + ones in the 65th column (softmax denominator source)
        bv_row = np.zeros(260, np.float32)
        for i, h in enumerate(hs):
            bv_row[i * 65 : i * 65 + 64] = b_attn[
                2 * D + h * HD : 2 * D + (h + 1) * HD
            ]
            bv_row[i * 65 + 64] = 1.0
        bv = np.broadcast_to(bv_row, (128, 260)).copy()
        # local 256 rows of w_proj as [128, pair, D]
        wrows = np.concatenate(
            [w_proj[h * HD : (h + 1) * HD, :] for h in hs], axis=0
        )  # [256, D]
        wpc = np.ascontiguousarray(
            wrows.reshape(NPAIR, 128, D).transpose(1, 0, 2)
        )
        # bias added on exactly one core per batch (exact: x + 0 == x)
        if g == 0:
            bpc = np.broadcast_to(b_proj, (128, D)).copy()
        else:
            bpc = np.zeros((128, D), np.float32)
        in_maps.append(
            dict(
                xt=xt, wv=wv, bqk=bqk, bv=bv, wp=wpc, bp=bpc, msk=msk,
                **wqk_t,
            )
        )
    return in_maps


def _unshard(results):
    y = np.empty((B, S, D), np.float32)
    for b in range(B):
        acc = results[b * GROUP]["y"].astype(np.float32, copy=True)
        for g in range(1, GROUP):
            acc += results[b * GROUP + g]["y"]
        y[b] = acc
    return y


_NC_CACHE = {}


def kernel(x, w_attn, b_attn, w_proj, b_proj):
    from concourse.bass_utils import run_bass_kernel_spmd

    if S not in _NC_CACHE:
        _NC_CACHE[S] = _build_bass(S)
    nc = _NC_CACHE[S]
    in_maps = _shard_inputs(x, w_attn, b_attn, w_proj, b_proj)
    res = run_bass_kernel_spmd(nc, in_maps, list(range(N_CORES)))
    return _unshard(res.results)


# revision 4
# speedup vs baseline: 1.0631x; 1.0631x over previous
"""Trainium2 Bass kernel for a GPT-style causal attention block.

  y = proj( softmax_causal( (x@Wq)(x@Wk)^T / sqrt(hd) ) @ (x@Wv) )

Shapes (hardcoded): B=2, S=2048, D=1024, H=16 heads, hd=64.

Sharding over 8 NeuronCores: core = (batch b, head-group g), g selects
4 heads. Row-parallel output projection: each core computes a PARTIAL
y[b] = a_g @ Wp[rows of g's heads, :] for the full D=1024 columns; the
host unshard sums the 4 partials per batch and adds b_proj (standard
Megatron row-parallel sharding). No on-device collective -> no
NEFF-start CC barrier (~46us) and no AllGather (~60us) on the critical
path, and no bf16 payload rounding (rel err ~2.6e-4 vs ~2.4e-3 for the
AllGather variant).

Per core (all matmuls float32r = full PE rate at moving dim >=256):

  inputs: host pre-arranges every tensor into its exact SBUF layout so
  each DMA is partition-major with >=2KB contiguous lines, split
  across both HWDGE queues (SP + ACT) in first-needed-first order.

  per query block ib (IB=512), pair-sequential:
    - q/k col-projections for just this block (lhsT = wqk col-group,
      rhs = xT slab) -> qT/kT [head_ch, S]; attention on block ib then
      runs while later xt slabs are still in flight.
    - v for key tiles <= ib on demand, natural [S, head_ch]; its 65th
      column per head is 0*x + bias(=1) -> the softmax-denominator
      ones column costs no DMA and no extra matmul.
    - scores in the transposed [key, query] layout, head PAIRS packed
      into one PE pass via row-group tile_position (0,0)/(64,0); for
      diagonal key tiles the fully-masked leading query columns are
      skipped in scores/exp/mask/AV (causal trim); exp on ACT with
      scale=1/8 folded in, per psum-bank half so AV(h) unblocks as
      soon as its half is ready; causal mask multiplies only diagonal
      tiles (DVE, emitted first to overlap the non-diagonal stream).
    - AV with lhsT=[v|1]: psum row 64 accumulates the denominator.
      normalize: denominator row -> SBUF (ACT copy),
      reciprocal_approx_fast (single custom-DVE op, ~18 bits), gpsimd
      partition_broadcast across the 64 head channels, one DVE mult
      -> aT [head_ch, S] f32r.
    - pair 1 only: projection of seq chunk ib-1 (delayed one block so
      these matmuls fill the PE while the normalize chain of the
      current block completes): psum[128,1024] accumulates both
      pairs' aT tiles against wp rows; psum -> SBUF via ACT copy
      (bias is added host-side); y stores ride the idle SP queue
      except the last chunk, which splits across both queues to halve
      the final drain.

Timeline on HW (neuron-profile): ~245us per core; PE busy ~77%, ACT
~50%, DVE ~27%. The remaining PE idle is NEFF-start DMA latency
(~14us), ~1us semaphore hiccups at block boundaries, and the final
y-store drain (~10us).
"""

import numpy as np

B = 2
S = 2048
D = 1024
H = 16
HD = 64
HLOC = 4          # heads per core
NPAIR = 2         # head pairs per core
N_CORES = 8
GROUP = 4         # cores per batch
IB = 512          # query block width (matmul moving dim)
JT = 128          # key tile (psum partition dim)
SCALE = 1.0 / 8.0  # 1/sqrt(hd)


def _build_bass(s=S, repeat=1, phases="all"):
    """Build the SPMD Bass program (one NeuronCore's view)."""
    import concourse.bacc as bacc
    import concourse.mybir as mybir
    import concourse.tile as tile

    f32 = mybir.dt.float32
    f32r = mybir.dt.float32r
    Alu = mybir.AluOpType
    Act = mybir.ActivationFunctionType

    n_ib = s // IB           # query blocks
    n_st = s // 128          # 128-row sequence tiles
    n_dt = D // 128          # contraction tiles for D

    nc = bacc.Bacc(num_devices=N_CORES)

    # all big inputs host-pre-arranged to SBUF layout (partition-major,
    # >=2KB contiguous per partition per DMA)
    xt = nc.declare_dram_parameter("xt", [s // IB, 128, D // 128, IB], f32r, isOutput=False)
    wqks = [
        nc.declare_dram_parameter(f"wqk{t}", [128, D // 128, 128], f32r, isOutput=False)
        for t in range(4)
    ]
    # v weights with a zero column per head (65th); + bias ones column
    # makes v_sb's denominator-ones column without any DMA
    wv = nc.declare_dram_parameter("wv", [128, D // 128, 260], f32r, isOutput=False)
    bqk = nc.declare_dram_parameter("bqk", [128, 4], f32, isOutput=False)
    bv = nc.declare_dram_parameter("bv", [128, 260], f32, isOutput=False)
    # row-parallel projection: local 256 rows of w_proj, all D columns,
    # as [128, pair, D] contraction tiles
    wp = nc.declare_dram_parameter("wp", [128, NPAIR, D], f32r, isOutput=False)
    msk = nc.declare_dram_parameter("msk", [128, 4, IB], f32, isOutput=False)
    y = nc.declare_dram_parameter("y", [s, D], f32, isOutput=True)

    with tile.TileContext(nc) as tc:
        with (
            tc.tile_pool(name="const", bufs=1) as const,
            tc.tile_pool(name="persist", bufs=1) as persist,
            tc.tile_pool(name="p1in", bufs=1) as p1in,
        ):
            # ---- input DMA, first-needed-first ----
            # qkT(pair0) needs wqk + xt slab; v_for(0..4)+attention ib0
            # needs wv, msk, xt slab0. Everything else can land later.
            # two HWDGE queues (SP + ACT), loads issued in consumption
            # order: wqk col-group t=0 (q pair0), xt slab 0, t=2 (k
            # pair0), wv, remaining slabs/cols, then phase-3 weights.
            # Every transfer is contiguous per partition (>=2KB lines).
            wqk_sbs = [
                p1in.tile([128, n_dt, 128], f32r, name=f"wqk_sb{t}")
                for t in range(4)
            ]
            xt_sb = p1in.tile([128, n_dt, s], f32r)
            wv_sb = p1in.tile([128, n_dt, 260], f32r)

            nc.scalar.dma_start(out=wqk_sbs[0], in_=wqks[0][:, :, :])
            for sh in range(n_ib):
                # split each slab across the two queues (dt halves)
                nc.sync.dma_start(
                    out=xt_sb[:, 0 : n_dt // 2, sh * IB : (sh + 1) * IB],
                    in_=xt[sh, :, 0 : n_dt // 2, :],
                )
                nc.scalar.dma_start(
                    out=xt_sb[:, n_dt // 2 :, sh * IB : (sh + 1) * IB],
                    in_=xt[sh, :, n_dt // 2 :, :],
                )
                if sh == 0:
                    nc.scalar.dma_start(out=wqk_sbs[2], in_=wqks[2][:, :, :])
                    nc.scalar.dma_start(out=wv_sb, in_=wv[:, :, :])
                    bqk_sb = const.tile([128, 4], f32)
                    nc.sync.dma_start(out=bqk_sb, in_=bqk[:, :])
                    bv_sb = const.tile([128, 260], f32)
                    nc.sync.dma_start(out=bv_sb, in_=bv[:, :])
                if sh == 1:
                    msk_sb = const.tile([128, 4, IB], f32)
                    nc.sync.dma_start(out=msk_sb, in_=msk[:, :, :])
                    nc.scalar.dma_start(out=wqk_sbs[1], in_=wqks[1][:, :, :])
                    nc.scalar.dma_start(out=wqk_sbs[3], in_=wqks[3][:, :, :])
            wp_sb = const.tile([128, NPAIR, D], f32r)
            nc.scalar.dma_start(out=wp_sb, in_=wp[:, :, :])
            # dummy exp: pulls the ACT exp table load off the critical path
            warm_sb = const.tile([1, 1], f32)
            nc.scalar.activation(
                out=warm_sb, in_=bqk_sb[0:1, 0:1], func=Act.Exp, scale=0.0
            )

            for _rep in range(repeat):
             # persistent intermediates
             qT_sb = persist.tile([128, NPAIR, s], f32r, name="qT_sb")
             kT_sb = persist.tile([128, NPAIR, s], f32r)
             v_sb = persist.tile([128, n_st, HLOC, 65], f32r)
             aT_sb = persist.tile([128, NPAIR, s], f32r)

             # v natural: lhsT = xT tile [d, s-tile], rhs = Wv [d, 256]
             def v_for(st_lo, st_hi, pool):
                 for st in range(st_lo, st_hi):
                     psv = pool.tile([128, 260], f32, name="psv", tag="pss")
                     for dt in range(n_dt):
                         nc.tensor.matmul(
                             psv,
                             lhsT=(xt_sb[:, dt, st * 128 : (st + 1) * 128]),
                             rhs=(wv_sb[:, dt, :]),
                             start=(dt == 0),
                             stop=(dt == n_dt - 1),
                         )
                     # per-head 65th col: 0 (zero weight col) + 1 (bias)
                     nc.vector.tensor_tensor(
                         out=v_sb[:, st, :, :],
                         in0=psv.rearrange("p (h e) -> p h e", h=HLOC),
                         in1=bv_sb.rearrange("p (h e) -> p h e", h=HLOC),
                         op=Alu.add,
                     )

             def qkT_one(t, sb, pool):
                 # qT/kT: lhsT = W tile [d,c], rhs = xT [d, s-block]
                 # c-tile t: 0,1 = q pair0/1; 2,3 = k pair0/1
                 ps = pool.tile([128, IB], f32, name="ps", tag="pss")
                 for dt in range(n_dt):
                     nc.tensor.matmul(
                         ps,
                         lhsT=(wqk_sbs[t][:, dt, :]),
                         rhs=(xt_sb[:, dt, sb * IB : (sb + 1) * IB]),
                         start=(dt == 0),
                         stop=(dt == n_dt - 1),
                     )
                 dst = qT_sb if t < 2 else kT_sb
                 nc.vector.tensor_scalar_add(
                     out=dst[:, t % 2, sb * IB : (sb + 1) * IB],
                     in0=ps,
                     scalar1=bqk_sb[:, t : t + 1],
                 )

             def qkT_for(t, pool):
                 for sb in range(n_ib):
                     qkT_one(t, sb, pool)

             if phases == "p1":
                 with tc.tile_pool(name="ps_p1", bufs=2, space="PSUM") as ps_p1:
                     v_for(0, n_st, ps_p1)
                     for t in range(4):
                         qkT_for(t, ps_p1)
                 continue

             # ---- attention + interleaved projection ----
             with (
                 tc.tile_pool(name="ps_s", bufs=3, space="PSUM") as ps_s,
                 tc.tile_pool(name="ps_av", bufs=1, space="PSUM") as ps_av,
                 tc.tile_pool(name="pt", bufs=3) as ptpool,
                 tc.tile_pool(name="small", bufs=2) as small,
                 tc.tile_pool(name="yout", bufs=2) as yout,
             ):
              def proj_chunk(ib):
                 # ---- projection of seq tiles 4*ib..4*ib+4 ----
                 # emitted one ib late so the next block's scores keep
                 # the PE busy while this block's normalize chain runs
                 for st in range(4 * ib, 4 * ib + 4):
                     psy = ps_s.tile([128, D], f32, name="psy", tag="pss")
                     for pp in range(NPAIR):
                         for ch in range(2):  # psum-bank halves
                             nc.tensor.matmul(
                                 psy[:, ch * 512 : (ch + 1) * 512],
                                 lhsT=(aT_sb[
                                         :, pp, st * 128 : (st + 1) * 128
                                     ]
                                 ),
                                 rhs=(wp_sb[:, pp, ch * 512 : (ch + 1) * 512]),
                                 start=(pp == 0),
                                 stop=(pp == NPAIR - 1),
                             )
                     ysb = yout.tile([128, D], f32, name="ysb")
                     # pure copy on ACT (b_proj is added host-side in
                     # the unshard sum); keeps DVE free for the
                     # normalize chain
                     nc.scalar.activation(
                         out=ysb, in_=psy, func=Act.Copy, scale=1.0
                     )
                     # last chunk: split stores across both queues so
                     # the final drain halves; earlier chunks stay on
                     # the idle SP queue (ACT dispatch costs ~0.8us)
                     yeng = (
                         nc.scalar
                         if st >= 4 * (n_ib - 1) and st % 2 == 1
                         else nc.sync
                     )
                     yeng.dma_start(
                         out=y[st * 128 : (st + 1) * 128, :], in_=ysb
                     )

              for pair in range(NPAIR):
                 for ib in range(n_ib):
                     # q/k col-projections for just this block: attention
                     # on block ib then runs while later slabs still DMA
                     qkT_one(pair, ib, ps_s)
                     qkT_one(2 + pair, ib, ps_s)
                     if pair == 0:
                         # v quarter-blocks on demand
                         v_for(4 * ib, 4 * ib + 4, ps_s)
                     njt = 4 * (ib + 1)  # key tiles needed (j <= i)
                     avs = [
                         ps_av.tile([65, IB], f32, name=f"av{hh}", tag=f"av{hh}")
                         for hh in range(2)
                     ]
                     # diagonal key tiles first: their mask multiply then
                     # overlaps the long non-diagonal score/AV stream
                     jt_order = list(range(4 * ib, njt)) + list(range(4 * ib))
                     for jseq, jt in enumerate(jt_order):
                         pss = ps_s.tile([128, 2 * IB], f32, name="pss")
                         k = jt - 4 * ib
                         # diagonal tile k: query columns < 128*k are
                         # fully masked -- skip them in scores/exp/AV
                         lo = 128 * k if k > 0 else 0
                         for hh in range(2):
                             off = hh * 64
                             nc.tensor.matmul(
                                 pss[:, hh * IB + lo : (hh + 1) * IB],
                                 lhsT=(kT_sb[
                                         off : off + 64,
                                         pair,
                                         jt * 128 : (jt + 1) * 128,
                                     ]
                                 ),
                                 rhs=(qT_sb[
                                         off : off + 64,
                                         pair,
                                         ib * IB + lo : (ib + 1) * IB,
                                     ]
                                 ),
                                 start=True,
                                 stop=True,
                                 tile_position=(off, 0),
                             )
                         pt = ptpool.tile([128, 2 * IB], f32r, name="pt")
                         for hh in range(2):
                             # exp per psum-bank half: AV(hh) unblocks as
                             # soon as its own half is through ACT
                             nc.scalar.activation(
                                 out=pt[:, hh * IB + lo : (hh + 1) * IB],
                                 in_=pss[:, hh * IB + lo : (hh + 1) * IB],
                                 func=Act.Exp,
                                 scale=SCALE,
                             )
                             if k >= 0:  # diagonal tile: causal mask
                                 nc.vector.tensor_tensor(
                                     out=pt[:, hh * IB + lo : (hh + 1) * IB],
                                     in0=pt[:, hh * IB + lo : (hh + 1) * IB],
                                     in1=msk_sb[:, k, lo:],
                                     op=Alu.mult,
                                 )
                             nc.tensor.matmul(
                                 avs[hh][:, lo:],
                                 lhsT=(v_sb[:, jt, pair * 2 + hh, :]),
                                 rhs=(pt[:, hh * IB + lo : (hh + 1) * IB]),
                                 start=(jseq == 0),
                                 stop=(jseq == njt - 1),
                                 skip_group_check=(k > 0 or jseq == njt - 1),
                             )
                     # normalize per head: aT = av[0:64] * (1 / av[64]).
                     # reciprocal_approx_fast on the [1,512] denominator
                     # row (~18 correct bits, plenty for softmax denoms),
                     # then a stride-0 DMA replicates it across the 64
                     # partitions -- no ACT/PE/psum involvement at all.
                     for hh in range(2):
                         av = avs[hh]
                         off = hh * 64
                         den_sb = small.tile([1, IB], f32, name="den_sb")
                         nc.scalar.activation(
                             out=den_sb,
                             in_=av[64:65, :],
                             func=Act.Copy,
                             scale=1.0,
                         )
                         rec1 = small.tile([1, IB], f32, name="rec1")
                         nc.vector.reciprocal_approx_fast(
                             out=rec1, in_=den_sb
                         )
                         rec_rep = small.tile([64, IB], f32, name="rec_rep")
                         nc.gpsimd.partition_broadcast(rec_rep, rec1)
                         nc.vector.tensor_tensor(
                             out=aT_sb[
                                 off : off + 64, pair, ib * IB : (ib + 1) * IB
                             ],
                             in0=av[0:64, :],
                             in1=rec_rep,
                             op=Alu.mult,
                         )
                     if pair == 1 and phases != "attn" and ib > 0:
                         proj_chunk(ib - 1)
                 if pair == 1 and phases != "attn":
                     proj_chunk(n_ib - 1)

    nc.compile()
    return nc


def _shard_inputs(x, w_attn, b_attn, w_proj, b_proj, s=S):
    """Host-side sharding: build the per-core input maps."""
    x = np.asarray(x, dtype=np.float32)
    w_attn = np.asarray(w_attn, dtype=np.float32)
    b_attn = np.asarray(b_attn, dtype=np.float32)
    w_proj = np.asarray(w_proj, dtype=np.float32)
    b_proj = np.asarray(b_proj, dtype=np.float32)

    # causal mask tiles: msk[j, k, i] = 1.0 if i >= j + 128*k
    jj = np.arange(128)[:, None, None]
    kk = np.arange(4)[None, :, None]
    ii = np.arange(IB)[None, None, :]
    msk = (ii >= jj + 128 * kk).astype(np.float32)

    n_dt = D // 128
    in_maps = []
    for core in range(N_CORES):
        b, g = divmod(core, GROUP)
        hs = list(range(g * HLOC, (g + 1) * HLOC))
        # xt: [sh, 128, dt, IB] so each slab is one contiguous DMA
        xtT = np.ascontiguousarray(x[b].T)  # [D, S]
        xt = np.ascontiguousarray(
            xtT.reshape(n_dt, 128, S // IB, IB).transpose(2, 1, 0, 3)
        )
        qcols = np.concatenate(
            [w_attn[:, h * HD : (h + 1) * HD] for h in hs], axis=1
        )
        kcols = np.concatenate(
            [w_attn[:, D + h * HD : D + (h + 1) * HD] for h in hs], axis=1
        )
        # wqk col-groups t=0,1 (q pairs), 2,3 (k pairs): [128, dt, 128]
        wqk_t = {}
        for t in range(4):
            blk = (qcols if t < 2 else kcols)[:, (t % 2) * 128 : (t % 2) * 128 + 128]
            wqk_t[f"wqk{t}"] = np.ascontiguousarray(
                blk.reshape(n_dt, 128, 128).transpose(1, 0, 2)
            )
        # wv with a zero 65th column per head: [D, 260] -> [128, dt, 260]
        wv_full = np.zeros((D, 260), np.float32)
        for i, h in enumerate(hs):
            wv_full[:, i * 65 : i * 65 + 64] = w_attn[
                :, 2 * D + h * HD : 2 * D + (h + 1) * HD
            ]
        wv = np.ascontiguousarray(
            wv_full.reshape(n_dt, 128, 260).transpose(1, 0, 2)
        )
        bq = np.concatenate([b_attn[h * HD : (h + 1) * HD] for h in hs])
        bk = np.concatenate([b_attn[D + h * HD : D + (h + 1) * HD] for h in hs])
        bqk = np.concatenate([bq, bk]).reshape(4, 128).T.copy()  # [128, 4]
        # v bias + ones in the 65th column (softmax denominator source)
        bv_row = np.zeros(260, np.float32)
        for i, h in enumerate(hs):
            bv_row[i * 65 : i * 65 + 64] = b_attn[
                2 * D + h * HD : 2 * D + (h + 1) * HD
            ]
            bv_row[i * 65 + 64] = 1.0
        bv = np.broadcast_to(bv_row, (128, 260)).copy()
        # local 256 rows of w_proj as [128, pair, D]
        wrows = np.concatenate(
            [w_proj[h * HD : (h + 1) * HD, :] for h in hs], axis=0
        )  # [256, D]
        wpc = np.ascontiguousarray(
            wrows.reshape(NPAIR, 128, D).transpose(1, 0, 2)
        )
        in_maps.append(
            dict(
                xt=xt, wv=wv, bqk=bqk, bv=bv, wp=wpc, msk=msk,
                **wqk_t,
            )
        )
    return in_maps


def _unshard(results, b_proj):
    y = np.empty((B, S, D), np.float32)
    for b in range(B):
        acc = results[b * GROUP]["y"].astype(np.float32, copy=True)
        for g in range(1, GROUP):
            acc += results[b * GROUP + g]["y"]
        y[b] = acc + b_proj.astype(np.float32)
    return y


_NC_CACHE = {}


def kernel(x, w_attn, b_attn, w_proj, b_proj):
    from concourse.bass_utils import run_bass_kernel_spmd

    if S not in _NC_CACHE:
        _NC_CACHE[S] = _build_bass(S)
    nc = _NC_CACHE[S]
    in_maps = _shard_inputs(x, w_attn, b_attn, w_proj, b_proj)
    res = run_bass_kernel_spmd(nc, in_maps, list(range(N_CORES)))
    return _unshard(res.results, np.asarray(b_proj))


# revision 5
# speedup vs baseline: 1.0818x; 1.0176x over previous
"""Trainium2 Bass kernel for a GPT-style causal attention block.

  y = proj( softmax_causal( (x@Wq)(x@Wk)^T / sqrt(hd) ) @ (x@Wv) )

Shapes (hardcoded): B=2, S=2048, D=1024, H=16 heads, hd=64.

Sharding over 8 NeuronCores: core = (batch b, head-group g), g selects
4 heads. Row-parallel output projection: each core computes a PARTIAL
y[b] = a_g @ Wp[rows of g's heads, :] for the full D=1024 columns; the
host unshard sums the 4 partials per batch and adds b_proj (standard
Megatron row-parallel sharding). No on-device collective -> no
NEFF-start CC barrier (~46us) and no AllGather (~60us) on the critical
path, and no bf16 payload rounding (rel err ~2.6e-4 vs ~2.4e-3 for the
AllGather variant).

Per core (all matmuls float32r = full PE rate at moving dim >=256):

  inputs: host pre-arranges every tensor into its exact SBUF layout so
  each DMA is partition-major with >=2KB contiguous lines, split
  across both HWDGE queues (SP + ACT) in first-needed-first order.

  per query block ib (IB=512), pair-sequential:
    - q/k col-projections for just this block (lhsT = wqk col-group,
      rhs = xT slab) -> qT/kT [head_ch, S]; attention on block ib then
      runs while later xt slabs are still in flight.
    - v for key tiles <= ib on demand, natural [S, head_ch]; its 65th
      column per head is 0*x + bias(=1) -> the softmax-denominator
      ones column costs no DMA and no extra matmul.
    - scores in the transposed [key, query] layout, head PAIRS packed
      into one PE pass via row-group tile_position (0,0)/(64,0); for
      diagonal key tiles the fully-masked leading query columns are
      skipped in scores/exp/mask/AV (causal trim); exp on ACT with
      scale=1/8 folded in, per psum-bank half so AV(h) unblocks as
      soon as its half is ready; causal mask multiplies only diagonal
      tiles (DVE, emitted first to overlap the non-diagonal stream).
    - AV with lhsT=[v|1]: psum row 64 accumulates the denominator.
      normalize: denominator row -> SBUF (ACT copy),
      reciprocal_approx_fast (single custom-DVE op, ~18 bits), gpsimd
      partition_broadcast across the 64 head channels, one DVE mult
      -> aT [head_ch, S] f32r.
    - pair 1 only: projection of seq chunk ib-1 (delayed one block so
      these matmuls fill the PE while the normalize chain of the
      current block completes): psum[128,1024] accumulates both
      pairs' aT tiles against wp rows; psum -> SBUF via ACT copy
      (bias is added host-side); y stores ride the idle SP queue
      except the last chunk, which splits across both queues to halve
      the final drain.

Timeline on HW (neuron-profile): ~245us per core; PE busy ~77%, ACT
~50%, DVE ~27%. The remaining PE idle is NEFF-start DMA latency
(~14us), ~1us semaphore hiccups at block boundaries, and the final
y-store drain (~10us).
"""

import numpy as np

B = 2
S = 2048
D = 1024
H = 16
HD = 64
HLOC = 4          # heads per core
NPAIR = 2         # head pairs per core
N_CORES = 8
GROUP = 4         # cores per batch
IB = 512          # query block width (matmul moving dim)
JT = 128          # key tile (psum partition dim)
SCALE = 1.0 / 8.0  # 1/sqrt(hd)


def _build_bass(s=S, repeat=1, phases="all"):
    """Build the SPMD Bass program (one NeuronCore's view)."""
    import concourse.bacc as bacc
    import concourse.mybir as mybir
    import concourse.tile as tile

    f32 = mybir.dt.float32
    f32r = mybir.dt.float32r
    Alu = mybir.AluOpType
    Act = mybir.ActivationFunctionType

    n_ib = s // IB           # query blocks
    n_st = s // 128          # 128-row sequence tiles
    n_dt = D // 128          # contraction tiles for D

    nc = bacc.Bacc(num_devices=N_CORES)

    # all big inputs host-pre-arranged to SBUF layout (partition-major,
    # >=2KB contiguous per partition per DMA)
    xt = nc.declare_dram_parameter("xt", [s // IB, 128, D // 128, IB], f32r, isOutput=False)
    wqks = [
        nc.declare_dram_parameter(f"wqk{t}", [128, D // 128, 128], f32r, isOutput=False)
        for t in range(4)
    ]
    # v weights with a zero column per head (65th); + bias ones column
    # makes v_sb's denominator-ones column without any DMA
    wv = nc.declare_dram_parameter("wv", [128, D // 128, 260], f32r, isOutput=False)
    bqk = nc.declare_dram_parameter("bqk", [128, 4], f32, isOutput=False)
    bv = nc.declare_dram_parameter("bv", [128, 260], f32, isOutput=False)
    # row-parallel projection: local 256 rows of w_proj, all D columns,
    # as [128, pair, D] contraction tiles
    wp = nc.declare_dram_parameter("wp", [128, NPAIR, D], f32r, isOutput=False)
    msk = nc.declare_dram_parameter("msk", [128, 4, IB], f32, isOutput=False)
    y = nc.declare_dram_parameter("y", [s, D], f32, isOutput=True)

    with tile.TileContext(nc) as tc:
        with (
            tc.tile_pool(name="const", bufs=1) as const,
            tc.tile_pool(name="persist", bufs=1) as persist,
            tc.tile_pool(name="p1in", bufs=1) as p1in,
        ):
            # ---- input DMA, first-needed-first ----
            # qkT(pair0) needs wqk + xt slab; v_for(0..4)+attention ib0
            # needs wv, msk, xt slab0. Everything else can land later.
            # two HWDGE queues (SP + ACT), loads issued in consumption
            # order: wqk col-group t=0 (q pair0), xt slab 0, t=2 (k
            # pair0), wv, remaining slabs/cols, then phase-3 weights.
            # Every transfer is contiguous per partition (>=2KB lines).
            wqk_sbs = [
                p1in.tile([128, n_dt, 128], f32r, name=f"wqk_sb{t}")
                for t in range(4)
            ]
            xt_sb = p1in.tile([128, n_dt, s], f32r)
            wv_sb = p1in.tile([128, n_dt, 260], f32r)

            nc.scalar.dma_start(out=wqk_sbs[0], in_=wqks[0][:, :, :])
            for sh in range(n_ib):
                # split each slab across the two queues (dt halves)
                nc.sync.dma_start(
                    out=xt_sb[:, 0 : n_dt // 2, sh * IB : (sh + 1) * IB],
                    in_=xt[sh, :, 0 : n_dt // 2, :],
                )
                nc.scalar.dma_start(
                    out=xt_sb[:, n_dt // 2 :, sh * IB : (sh + 1) * IB],
                    in_=xt[sh, :, n_dt // 2 :, :],
                )
                if sh == 0:
                    nc.scalar.dma_start(out=wqk_sbs[2], in_=wqks[2][:, :, :])
                    nc.scalar.dma_start(out=wv_sb, in_=wv[:, :, :])
                    bqk_sb = const.tile([128, 4], f32)
                    nc.sync.dma_start(out=bqk_sb, in_=bqk[:, :])
                    bv_sb = const.tile([128, 260], f32)
                    nc.sync.dma_start(out=bv_sb, in_=bv[:, :])
                if sh == 1:
                    msk_sb = const.tile([128, 4, IB], f32)
                    nc.sync.dma_start(out=msk_sb, in_=msk[:, :, :])
                    nc.scalar.dma_start(out=wqk_sbs[1], in_=wqks[1][:, :, :])
                    nc.scalar.dma_start(out=wqk_sbs[3], in_=wqks[3][:, :, :])
            wp_sb = const.tile([128, NPAIR, D], f32r)
            nc.scalar.dma_start(out=wp_sb, in_=wp[:, :, :])
            # dummy exp: pulls the ACT exp table load off the critical path
            warm_sb = const.tile([1, 1], f32)
            nc.scalar.activation(
                out=warm_sb, in_=bqk_sb[0:1, 0:1], func=Act.Exp, scale=0.0
            )

            for _rep in range(repeat):
             # persistent intermediates
             qT_sb = persist.tile([128, NPAIR, s], f32r, name="qT_sb")
             kT_sb = persist.tile([128, NPAIR, s], f32r)
             v_sb = persist.tile([128, n_st, HLOC, 65], f32r)
             aT_sb = persist.tile([128, NPAIR, s], f32r)

             # v natural: lhsT = xT tile [d, s-tile], rhs = Wv [d, 256]
             def v_for(st_lo, st_hi, pool):
                 for st in range(st_lo, st_hi):
                     psv = pool.tile([128, 260], f32, name="psv", tag="pss")
                     for dt in range(n_dt):
                         nc.tensor.matmul(
                             psv,
                             lhsT=(xt_sb[:, dt, st * 128 : (st + 1) * 128]),
                             rhs=(wv_sb[:, dt, :]),
                             start=(dt == 0),
                             stop=(dt == n_dt - 1),
                         )
                     # per-head 65th col: 0 (zero weight col) + 1 (bias)
                     nc.vector.tensor_tensor(
                         out=v_sb[:, st, :, :],
                         in0=psv.rearrange("p (h e) -> p h e", h=HLOC),
                         in1=bv_sb.rearrange("p (h e) -> p h e", h=HLOC),
                         op=Alu.add,
                     )

             def qkT_one(t, sb, pool):
                 # qT/kT: lhsT = W tile [d,c], rhs = xT [d, s-block]
                 # c-tile t: 0,1 = q pair0/1; 2,3 = k pair0/1
                 ps = pool.tile([128, IB], f32, name="ps", tag="pss")
                 for dt in range(n_dt):
                     nc.tensor.matmul(
                         ps,
                         lhsT=(wqk_sbs[t][:, dt, :]),
                         rhs=(xt_sb[:, dt, sb * IB : (sb + 1) * IB]),
                         start=(dt == 0),
                         stop=(dt == n_dt - 1),
                     )
                 dst = qT_sb if t < 2 else kT_sb
                 nc.vector.tensor_scalar_add(
                     out=dst[:, t % 2, sb * IB : (sb + 1) * IB],
                     in0=ps,
                     scalar1=bqk_sb[:, t : t + 1],
                 )

             def qkT_for(t, pool):
                 for sb in range(n_ib):
                     qkT_one(t, sb, pool)

             if phases == "p1":
                 with tc.tile_pool(name="ps_p1", bufs=2, space="PSUM") as ps_p1:
                     v_for(0, n_st, ps_p1)
                     for t in range(4):
                         qkT_for(t, ps_p1)
                 continue

             # ---- attention + interleaved projection ----
             with (
                 tc.tile_pool(name="ps_s", bufs=3, space="PSUM") as ps_s,
                 tc.tile_pool(name="ps_av", bufs=1, space="PSUM") as ps_av,
                 tc.tile_pool(name="pt", bufs=3) as ptpool,
                 tc.tile_pool(name="small", bufs=2) as small,
                 tc.tile_pool(name="yout", bufs=2) as yout,
             ):
              def proj_chunk(ib):
                 # ---- projection of seq tiles 4*ib..4*ib+4 ----
                 # emitted one ib late so the next block's scores keep
                 # the PE busy while this block's normalize chain runs
                 for st in range(4 * ib, 4 * ib + 4):
                     psy = ps_s.tile([128, D], f32, name="psy", tag="pss")
                     for pp in range(NPAIR):
                         for ch in range(2):  # psum-bank halves
                             nc.tensor.matmul(
                                 psy[:, ch * 512 : (ch + 1) * 512],
                                 lhsT=(aT_sb[
                                         :, pp, st * 128 : (st + 1) * 128
                                     ]
                                 ),
                                 rhs=(wp_sb[:, pp, ch * 512 : (ch + 1) * 512]),
                                 start=(pp == 0),
                                 stop=(pp == NPAIR - 1),
                             )
                     ysb = yout.tile([128, D], f32, name="ysb")
                     # pure copy on ACT (b_proj is added host-side in
                     # the unshard sum); keeps DVE free for the
                     # normalize chain
                     nc.scalar.activation(
                         out=ysb, in_=psy, func=Act.Copy, scale=1.0
                     )
                     # last chunk: split stores across both queues so
                     # the final drain halves; earlier chunks stay on
                     # the idle SP queue (ACT dispatch costs ~0.8us)
                     yeng = (
                         nc.scalar
                         if st >= 4 * (n_ib - 1) and st % 2 == 1
                         else nc.sync
                     )
                     yeng.dma_start(
                         out=y[st * 128 : (st + 1) * 128, :], in_=ysb
                     )

              for pair in range(NPAIR):
                 for ib in range(n_ib):
                     # q/k col-projections for just this block: attention
                     # on block ib then runs while later slabs still DMA
                     qkT_one(pair, ib, ps_s)
                     qkT_one(2 + pair, ib, ps_s)
                     if pair == 0:
                         # v quarter-blocks on demand
                         v_for(4 * ib, 4 * ib + 4, ps_s)
                     njt = 4 * (ib + 1)  # key tiles needed (j <= i)
                     avs = [
                         ps_av.tile([65, IB], f32, name=f"av{hh}", tag=f"av{hh}")
                         for hh in range(2)
                     ]
                     # diagonal key tiles first: their mask multiply then
                     # overlaps the long non-diagonal score/AV stream
                     jt_order = list(range(4 * ib, njt)) + list(range(4 * ib))

                     def emit_scores(jseq, jt):
                         # diagonal tile k: query columns < 128*k are
                         # fully masked -- skip them in scores/exp/AV
                         k = jt - 4 * ib
                         lo = 128 * k if k > 0 else 0
                         pss = ps_s.tile([128, 2 * IB], f32, name="pss")
                         for hh in range(2):
                             off = hh * 64
                             nc.tensor.matmul(
                                 pss[:, hh * IB + lo : (hh + 1) * IB],
                                 lhsT=(kT_sb[
                                         off : off + 64,
                                         pair,
                                         jt * 128 : (jt + 1) * 128,
                                     ]
                                 ),
                                 rhs=(qT_sb[
                                         off : off + 64,
                                         pair,
                                         ib * IB + lo : (ib + 1) * IB,
                                     ]
                                 ),
                                 start=True,
                                 stop=True,
                                 tile_position=(off, 0),
                             )
                         pt = ptpool.tile([128, 2 * IB], f32r, name="pt")
                         for hh in range(2):
                             # exp per psum-bank half: AV(hh) unblocks as
                             # soon as its own half is through ACT
                             nc.scalar.activation(
                                 out=pt[:, hh * IB + lo : (hh + 1) * IB],
                                 in_=pss[:, hh * IB + lo : (hh + 1) * IB],
                                 func=Act.Exp,
                                 scale=SCALE,
                             )
                             if k >= 0:  # diagonal tile: causal mask
                                 nc.vector.tensor_tensor(
                                     out=pt[:, hh * IB + lo : (hh + 1) * IB],
                                     in0=pt[:, hh * IB + lo : (hh + 1) * IB],
                                     in1=msk_sb[:, k, lo:],
                                     op=Alu.mult,
                                 )
                         return pt, lo

                     def emit_av(jseq, jt, pt, lo):
                         for hh in range(2):
                             nc.tensor.matmul(
                                 avs[hh][:, lo:],
                                 lhsT=(v_sb[:, jt, pair * 2 + hh, :]),
                                 rhs=(pt[:, hh * IB + lo : (hh + 1) * IB]),
                                 start=(jseq == 0),
                                 stop=(jseq == njt - 1),
                                 skip_group_check=True,
                             )

                     # software-pipeline the PE queue by one stage: the
                     # engine executes its queue in order, so scores of
                     # jt+1 must be enqueued BEFORE AV of jt or a stall
                     # on exp(jt) also blocks the (ready) next scores
                     pend = None
                     for jseq, jt in enumerate(jt_order):
                         pt, lo = emit_scores(jseq, jt)
                         if pend is not None:
                             emit_av(*pend)
                         pend = (jseq, jt, pt, lo)
                     emit_av(*pend)
                     # normalize per head: aT = av[0:64] * (1 / av[64]).
                     # reciprocal_approx_fast on the [1,512] denominator
                     # row (~18 correct bits, plenty for softmax denoms),
                     # then a stride-0 DMA replicates it across the 64
                     # partitions -- no ACT/PE/psum involvement at all.
                     for hh in range(2):
                         av = avs[hh]
                         off = hh * 64
                         den_sb = small.tile([1, IB], f32, name="den_sb")
                         nc.scalar.activation(
                             out=den_sb,
                             in_=av[64:65, :],
                             func=Act.Copy,
                             scale=1.0,
                         )
                         rec1 = small.tile([1, IB], f32, name="rec1")
                         nc.vector.reciprocal_approx_fast(
                             out=rec1, in_=den_sb
                         )
                         rec_rep = small.tile([64, IB], f32, name="rec_rep")
                         nc.gpsimd.partition_broadcast(rec_rep, rec1)
                         nc.vector.tensor_tensor(
                             out=aT_sb[
                                 off : off + 64, pair, ib * IB : (ib + 1) * IB
                             ],
                             in0=av[0:64, :],
                             in1=rec_rep,
                             op=Alu.mult,
                         )
                     if pair == 1 and phases != "attn" and ib > 0:
                         proj_chunk(ib - 1)
                 if pair == 1 and phases != "attn":
                     proj_chunk(n_ib - 1)

    nc.compile()
    return nc


def _shard_inputs(x, w_attn, b_attn, w_proj, b_proj, s=S):
    """Host-side sharding: build the per-core input maps."""
    x = np.asarray(x, dtype=np.float32)
    w_attn = np.asarray(w_attn, dtype=np.float32)
    b_attn = np.asarray(b_attn, dtype=np.float32)
    w_proj = np.asarray(w_proj, dtype=np.float32)
    b_proj = np.asarray(b_proj, dtype=np.float32)

    # causal mask tiles: msk[j, k, i] = 1.0 if i >= j + 128*k
    jj = np.arange(128)[:, None, None]
    kk = np.arange(4)[None, :, None]
    ii = np.arange(IB)[None, None, :]
    msk = (ii >= jj + 128 * kk).astype(np.float32)

    n_dt = D // 128
    in_maps = []
    for core in range(N_CORES):
        b, g = divmod(core, GROUP)
        hs = list(range(g * HLOC, (g + 1) * HLOC))
        # xt: [sh, 128, dt, IB] so each slab is one contiguous DMA
        xtT = np.ascontiguousarray(x[b].T)  # [D, S]
        xt = np.ascontiguousarray(
            xtT.reshape(n_dt, 128, S // IB, IB).transpose(2, 1, 0, 3)
        )
        qcols = np.concatenate(
            [w_attn[:, h * HD : (h + 1) * HD] for h in hs], axis=1
        )
        kcols = np.concatenate(
            [w_attn[:, D + h * HD : D + (h + 1) * HD] for h in hs], axis=1
        )
        # wqk col-groups t=0,1 (q pairs), 2,3 (k pairs): [128, dt, 128]
        wqk_t = {}
        for t in range(4):
            blk = (qcols if t < 2 else kcols)[:, (t % 2) * 128 : (t % 2) * 128 + 128]
            wqk_t[f"wqk{t}"] = np.ascontiguousarray(
                blk.reshape(n_dt, 128, 128).transpose(1, 0, 2)
            )
        # wv with a zero 65th column per head: [D, 260] -> [128, dt, 260]
        wv_full = np.zeros((D, 260), np.float32)
        for i, h in enumerate(hs):
            wv_full[:, i * 65 : i * 65 + 64] = w_attn[
                :, 2 * D + h * HD : 2 * D + (h + 1) * HD
            ]
        wv = np.ascontiguousarray(
            wv_full.reshape(n_dt, 128, 260).transpose(1, 0, 2)
        )
        bq = np.concatenate([b_attn[h * HD : (h + 1) * HD] for h in hs])
        bk = np.concatenate([b_attn[D + h * HD : D + (h + 1) * HD] for h in hs])
        bqk = np.concatenate([bq, bk]).reshape(4, 128).T.copy()  # [128, 4]
        # v bias + ones in the 65th column (softmax denominator source)
        bv_row = np.zeros(260, np.float32)
        for i, h in enumerate(hs):
            bv_row[i * 65 : i * 65 + 64] = b_attn[
                2 * D + h * HD : 2 * D + (h + 1) * HD
            ]
            bv_row[i * 65 + 64] = 1.0
        bv = np.broadcast_to(bv_row, (128, 260)).copy()
        # local 256 rows of w_proj as [128, pair, D]
        wrows = np.concatenate(
            [w_proj[h * HD : (h + 1) * HD, :] for h in hs], axis=0
        )  # [256, D]
        wpc = np.ascontiguousarray(
            wrows.reshape(NPAIR, 128, D).transpose(1, 0, 2)
        )
        in_maps.append(
            dict(
                xt=xt, wv=wv, bqk=bqk, bv=bv, wp=wpc, msk=msk,
                **wqk_t,
            )
        )
    return in_maps


def _unshard(results, b_proj):
    y = np.empty((B, S, D), np.float32)
    for b in range(B):
        acc = results[b * GROUP]["y"].astype(np.float32, copy=True)
        for g in range(1, GROUP):
            acc += results[b * GROUP + g]["y"]
        y[b] = acc + b_proj.astype(np.float32)
    return y


_NC_CACHE = {}


def kernel(x, w_attn, b_attn, w_proj, b_proj):
    from concourse.bass_utils import run_bass_kernel_spmd

    if S not in _NC_CACHE:
        _NC_CACHE[S] = _build_bass(S)
    nc = _NC_CACHE[S]
    in_maps = _shard_inputs(x, w_attn, b_attn, w_proj, b_proj)
    res = run_bass_kernel_spmd(nc, in_maps, list(range(N_CORES)))
    return _unshard(res.results, np.asarray(b_proj))


# revision 6
# speedup vs baseline: 1.0912x; 1.0087x over previous
"""Trainium2 Bass kernel for a GPT-style causal attention block.

  y = proj( softmax_causal( (x@Wq)(x@Wk)^T / sqrt(hd) ) @ (x@Wv) )

Shapes (hardcoded): B=2, S=2048, D=1024, H=16 heads, hd=64.

Sharding over 8 NeuronCores: core = (batch b, head-group g), g selects
4 heads. Row-parallel output projection: each core computes a PARTIAL
y[b] = a_g @ Wp[rows of g's heads, :] for the full D=1024 columns; the
host unshard sums the 4 partials per batch and adds b_proj (standard
Megatron row-parallel sharding). No on-device collective -> no
NEFF-start CC barrier (~46us) and no AllGather (~60us) on the critical
path, and no bf16 payload rounding (rel err ~2.6e-4 vs ~2.4e-3 for the
AllGather variant).

Per core (all matmuls float32r = full PE rate at moving dim >=256):

  inputs: host pre-arranges every tensor into its exact SBUF layout so
  each DMA is partition-major with >=2KB contiguous lines, split
  across both HWDGE queues (SP + ACT) in first-needed-first order.

  per query block ib (IB=512), pair-sequential:
    - q/k col-projections for just this block (lhsT = wqk col-group,
      rhs = xT slab) -> qT/kT [head_ch, S]; attention on block ib then
      runs while later xt slabs are still in flight.
    - v for key tiles <= ib on demand, natural [S, head_ch]; its 65th
      column per head is 0*x + bias(=1) -> the softmax-denominator
      ones column costs no DMA and no extra matmul.
    - scores in the transposed [key, query] layout, head PAIRS packed
      into one PE pass via row-group tile_position (0,0)/(64,0); for
      diagonal key tiles the fully-masked leading query columns are
      skipped in scores/exp/mask/AV (causal trim); exp on ACT with
      scale=1/8 folded in, per psum-bank half so AV(h) unblocks as
      soon as its half is ready; causal mask multiplies only diagonal
      tiles (DVE, emitted first to overlap the non-diagonal stream).
    - AV with lhsT=[v|1]: psum row 64 accumulates the denominator.
      normalize: denominator row -> SBUF (ACT copy),
      reciprocal_approx_fast (single custom-DVE op, ~18 bits), gpsimd
      partition_broadcast across the 64 head channels, one DVE mult
      -> aT [head_ch, S] f32r.
    - pair 1 only: projection of seq chunk ib-1 (delayed one block so
      these matmuls fill the PE while the normalize chain of the
      current block completes): psum[128,1024] accumulates both
      pairs' aT tiles against wp rows; psum -> SBUF via ACT copy
      (bias is added host-side); y stores ride the idle SP queue
      except the last chunk, which splits across both queues to halve
      the final drain.

Timeline on HW (neuron-profile): ~245us per core; PE busy ~77%, ACT
~50%, DVE ~27%. The remaining PE idle is NEFF-start DMA latency
(~14us), ~1us semaphore hiccups at block boundaries, and the final
y-store drain (~10us).
"""

import numpy as np

B = 2
S = 2048
D = 1024
H = 16
HD = 64
HLOC = 4          # heads per core
NPAIR = 2         # head pairs per core
N_CORES = 8
GROUP = 4         # cores per batch
IB = 512          # query block width (matmul moving dim)
JT = 128          # key tile (psum partition dim)
SCALE = 1.0 / 8.0  # 1/sqrt(hd)


def _build_bass(s=S, repeat=1, phases="all"):
    """Build the SPMD Bass program (one NeuronCore's view)."""
    import concourse.bacc as bacc
    import concourse.mybir as mybir
    import concourse.tile as tile

    f32 = mybir.dt.float32
    f32r = mybir.dt.float32r
    Alu = mybir.AluOpType
    Act = mybir.ActivationFunctionType

    n_ib = s // IB           # query blocks
    n_st = s // 128          # 128-row sequence tiles
    n_dt = D // 128          # contraction tiles for D

    nc = bacc.Bacc(num_devices=N_CORES)

    # all big inputs host-pre-arranged to SBUF layout (partition-major,
    # >=2KB contiguous per partition per DMA)
    xt = nc.declare_dram_parameter("xt", [s // IB, 128, D // 128, IB], f32r, isOutput=False)
    wqks = [
        nc.declare_dram_parameter(f"wqk{t}", [128, D // 128, 128], f32r, isOutput=False)
        for t in range(4)
    ]
    # v weights with a zero column per head (65th); + bias ones column
    # makes v_sb's denominator-ones column without any DMA
    wv = nc.declare_dram_parameter("wv", [128, D // 128, 260], f32r, isOutput=False)
    bqk = nc.declare_dram_parameter("bqk", [128, 4], f32, isOutput=False)
    bv = nc.declare_dram_parameter("bv", [128, 260], f32, isOutput=False)
    # row-parallel projection: local 256 rows of w_proj, all D columns,
    # as [128, pair, D] contraction tiles
    wp = nc.declare_dram_parameter("wp", [128, NPAIR, D], f32r, isOutput=False)
    msk = nc.declare_dram_parameter("msk", [128, 4, IB], f32, isOutput=False)
    y = nc.declare_dram_parameter("y", [s, D], f32, isOutput=True)

    with tile.TileContext(nc) as tc:
        with (
            tc.tile_pool(name="const", bufs=1) as const,
            tc.tile_pool(name="persist", bufs=1) as persist,
            tc.tile_pool(name="p1in", bufs=1) as p1in,
        ):
            # ---- input DMA, first-needed-first ----
            # qkT(pair0) needs wqk + xt slab; v_for(0..4)+attention ib0
            # needs wv, msk, xt slab0. Everything else can land later.
            # two HWDGE queues (SP + ACT), loads issued in consumption
            # order: wqk col-group t=0 (q pair0), xt slab 0, t=2 (k
            # pair0), wv, remaining slabs/cols, then phase-3 weights.
            # Every transfer is contiguous per partition (>=2KB lines).
            wqk_sbs = [
                p1in.tile([128, n_dt, 128], f32r, name=f"wqk_sb{t}")
                for t in range(4)
            ]
            xt_sb = p1in.tile([128, n_dt, s], f32r)
            wv_sb = p1in.tile([128, n_dt, 260], f32r)

            nc.scalar.dma_start(out=wqk_sbs[0], in_=wqks[0][:, :, :])
            for sh in range(n_ib):
                # split each slab across the two queues (dt halves);
                # slab 0 additionally in quarters so the first qkT
                # accumulation can start streaming ~2us sooner
                nq = 4 if sh == 0 else 2
                for qi in range(nq):
                    d0 = qi * n_dt // nq
                    d1 = (qi + 1) * n_dt // nq
                    eng = nc.sync if qi < nq // 2 else nc.scalar
                    eng.dma_start(
                        out=xt_sb[:, d0:d1, sh * IB : (sh + 1) * IB],
                        in_=xt[sh, :, d0:d1, :],
                    )
                if sh == 0:
                    nc.scalar.dma_start(out=wqk_sbs[2], in_=wqks[2][:, :, :])
                    nc.scalar.dma_start(out=wv_sb, in_=wv[:, :, :])
                    bqk_sb = const.tile([128, 4], f32)
                    nc.sync.dma_start(out=bqk_sb, in_=bqk[:, :])
                    bv_sb = const.tile([128, 260], f32)
                    nc.sync.dma_start(out=bv_sb, in_=bv[:, :])
                if sh == 1:
                    msk_sb = const.tile([128, 4, IB], f32)
                    nc.sync.dma_start(out=msk_sb, in_=msk[:, :, :])
                    nc.scalar.dma_start(out=wqk_sbs[1], in_=wqks[1][:, :, :])
                    nc.scalar.dma_start(out=wqk_sbs[3], in_=wqks[3][:, :, :])
            wp_sb = const.tile([128, NPAIR, D], f32r)
            nc.scalar.dma_start(out=wp_sb, in_=wp[:, :, :])
            # dummy exp: pulls the ACT exp table load off the critical path
            warm_sb = const.tile([1, 1], f32)
            nc.scalar.activation(
                out=warm_sb, in_=bqk_sb[0:1, 0:1], func=Act.Exp, scale=0.0
            )

            for _rep in range(repeat):
             # persistent intermediates
             qT_sb = persist.tile([128, NPAIR, s], f32r, name="qT_sb")
             kT_sb = persist.tile([128, NPAIR, s], f32r)
             v_sb = persist.tile([128, n_st, HLOC, 65], f32r)
             aT_sb = persist.tile([128, NPAIR, s], f32r)

             # v natural: lhsT = xT tile [d, s-tile], rhs = Wv [d, 256]
             def v_for(st_lo, st_hi, pool):
                 for st in range(st_lo, st_hi):
                     psv = pool.tile([128, 260], f32, name="psv", tag="pss")
                     for dt in range(n_dt):
                         nc.tensor.matmul(
                             psv,
                             lhsT=(xt_sb[:, dt, st * 128 : (st + 1) * 128]),
                             rhs=(wv_sb[:, dt, :]),
                             start=(dt == 0),
                             stop=(dt == n_dt - 1),
                         )
                     # per-head 65th col: 0 (zero weight col) + 1 (bias)
                     nc.vector.tensor_tensor(
                         out=v_sb[:, st, :, :],
                         in0=psv.rearrange("p (h e) -> p h e", h=HLOC),
                         in1=bv_sb.rearrange("p (h e) -> p h e", h=HLOC),
                         op=Alu.add,
                     )

             def qkT_one(t, sb, pool):
                 # qT/kT: lhsT = W tile [d,c], rhs = xT [d, s-block]
                 # c-tile t: 0,1 = q pair0/1; 2,3 = k pair0/1
                 ps = pool.tile([128, IB], f32, name="ps", tag="pss")
                 for dt in range(n_dt):
                     nc.tensor.matmul(
                         ps,
                         lhsT=(wqk_sbs[t][:, dt, :]),
                         rhs=(xt_sb[:, dt, sb * IB : (sb + 1) * IB]),
                         start=(dt == 0),
                         stop=(dt == n_dt - 1),
                     )
                 dst = qT_sb if t < 2 else kT_sb
                 nc.vector.tensor_scalar_add(
                     out=dst[:, t % 2, sb * IB : (sb + 1) * IB],
                     in0=ps,
                     scalar1=bqk_sb[:, t : t + 1],
                 )

             def qkT_for(t, pool):
                 for sb in range(n_ib):
                     qkT_one(t, sb, pool)

             if phases == "p1":
                 with tc.tile_pool(name="ps_p1", bufs=2, space="PSUM") as ps_p1:
                     v_for(0, n_st, ps_p1)
                     for t in range(4):
                         qkT_for(t, ps_p1)
                 continue

             # ---- attention + interleaved projection ----
             with (
                 tc.tile_pool(name="ps_s", bufs=3, space="PSUM") as ps_s,
                 tc.tile_pool(name="ps_av", bufs=1, space="PSUM") as ps_av,
                 tc.tile_pool(name="pt", bufs=3) as ptpool,
                 tc.tile_pool(name="small", bufs=2) as small,
                 tc.tile_pool(name="yout", bufs=2) as yout,
             ):
              def proj_chunk(ib):
                 # ---- projection of seq tiles 4*ib..4*ib+4 ----
                 # emitted one ib late so the next block's scores keep
                 # the PE busy while this block's normalize chain runs
                 for st in range(4 * ib, 4 * ib + 4):
                     psy = ps_s.tile([128, D], f32, name="psy", tag="pss")
                     for pp in range(NPAIR):
                         for ch in range(2):  # psum-bank halves
                             nc.tensor.matmul(
                                 psy[:, ch * 512 : (ch + 1) * 512],
                                 lhsT=(aT_sb[
                                         :, pp, st * 128 : (st + 1) * 128
                                     ]
                                 ),
                                 rhs=(wp_sb[:, pp, ch * 512 : (ch + 1) * 512]),
                                 start=(pp == 0),
                                 stop=(pp == NPAIR - 1),
                             )
                     ysb = yout.tile([128, D], f32, name="ysb")
                     # pure copy (b_proj is added host-side in the
                     # unshard sum); DVE, off the exp-critical ACT queue
                     nc.vector.tensor_copy(out=ysb, in_=psy)
                     # last chunk: split stores across both queues so
                     # the final drain halves; earlier chunks stay on
                     # the idle SP queue (ACT dispatch costs ~0.8us)
                     yeng = (
                         nc.scalar
                         if st >= 4 * (n_ib - 1) and st % 2 == 1
                         else nc.sync
                     )
                     yeng.dma_start(
                         out=y[st * 128 : (st + 1) * 128, :], in_=ysb
                     )

              for pair in range(NPAIR):
                 for ib in range(n_ib):
                     # q/k col-projections for just this block: attention
                     # on block ib then runs while later slabs still DMA
                     qkT_one(pair, ib, ps_s)
                     qkT_one(2 + pair, ib, ps_s)
                     if pair == 0:
                         # v quarter-blocks on demand
                         v_for(4 * ib, 4 * ib + 4, ps_s)
                     njt = 4 * (ib + 1)  # key tiles needed (j <= i)
                     avs = [
                         ps_av.tile([65, IB], f32, name=f"av{hh}", tag=f"av{hh}")
                         for hh in range(2)
                     ]
                     # diagonal key tiles first: their mask multiply then
                     # overlaps the long non-diagonal score/AV stream
                     jt_order = list(range(4 * ib, njt)) + list(range(4 * ib))

                     def emit_scores(jseq, jt):
                         # diagonal tile k: query columns < 128*k are
                         # fully masked -- skip them in scores/exp/AV
                         k = jt - 4 * ib
                         lo = 128 * k if k > 0 else 0
                         pss = ps_s.tile([128, 2 * IB], f32, name="pss")
                         for hh in range(2):
                             off = hh * 64
                             nc.tensor.matmul(
                                 pss[:, hh * IB + lo : (hh + 1) * IB],
                                 lhsT=(kT_sb[
                                         off : off + 64,
                                         pair,
                                         jt * 128 : (jt + 1) * 128,
                                     ]
                                 ),
                                 rhs=(qT_sb[
                                         off : off + 64,
                                         pair,
                                         ib * IB + lo : (ib + 1) * IB,
                                     ]
                                 ),
                                 start=True,
                                 stop=True,
                                 tile_position=(off, 0),
                             )
                         pt = ptpool.tile([128, 2 * IB], f32r, name="pt")
                         for hh in range(2):
                             # exp per psum-bank half: AV(hh) unblocks as
                             # soon as its own half is through ACT
                             nc.scalar.activation(
                                 out=pt[:, hh * IB + lo : (hh + 1) * IB],
                                 in_=pss[:, hh * IB + lo : (hh + 1) * IB],
                                 func=Act.Exp,
                                 scale=SCALE,
                             )
                             if k >= 0:  # diagonal tile: causal mask
                                 nc.vector.tensor_tensor(
                                     out=pt[:, hh * IB + lo : (hh + 1) * IB],
                                     in0=pt[:, hh * IB + lo : (hh + 1) * IB],
                                     in1=msk_sb[:, k, lo:],
                                     op=Alu.mult,
                                 )
                         return pt, lo

                     def emit_av(jseq, jt, pt, lo):
                         for hh in range(2):
                             nc.tensor.matmul(
                                 avs[hh][:, lo:],
                                 lhsT=(v_sb[:, jt, pair * 2 + hh, :]),
                                 rhs=(pt[:, hh * IB + lo : (hh + 1) * IB]),
                                 start=(jseq == 0),
                                 stop=(jseq == njt - 1),
                                 skip_group_check=True,
                             )

                     # software-pipeline the PE queue by one stage: the
                     # engine executes its queue in order, so scores of
                     # jt+1 must be enqueued BEFORE AV of jt or a stall
                     # on exp(jt) also blocks the (ready) next scores
                     pend = None
                     for jseq, jt in enumerate(jt_order):
                         pt, lo = emit_scores(jseq, jt)
                         if pend is not None:
                             emit_av(*pend)
                         pend = (jseq, jt, pt, lo)
                     emit_av(*pend)
                     # normalize per head: aT = av[0:64] * (1 / av[64]).
                     # reciprocal_approx_fast on the [1,512] denominator
                     # row (~18 correct bits, plenty for softmax denoms),
                     # then a stride-0 DMA replicates it across the 64
                     # partitions -- no ACT/PE/psum involvement at all.
                     for hh in range(2):
                         av = avs[hh]
                         off = hh * 64
                         den_sb = small.tile([1, IB], f32, name="den_sb")
                         nc.scalar.activation(
                             out=den_sb,
                             in_=av[64:65, :],
                             func=Act.Copy,
                             scale=1.0,
                         )
                         rec1 = small.tile([1, IB], f32, name="rec1")
                         nc.vector.reciprocal_approx_fast(
                             out=rec1, in_=den_sb
                         )
                         rec_rep = small.tile([64, IB], f32, name="rec_rep")
                         nc.gpsimd.partition_broadcast(rec_rep, rec1)
                         nc.vector.tensor_tensor(
                             out=aT_sb[
                                 off : off + 64, pair, ib * IB : (ib + 1) * IB
                             ],
                             in0=av[0:64, :],
                             in1=rec_rep,
                             op=Alu.mult,
                         )
                     if pair == 1 and phases != "attn" and ib > 0:
                         proj_chunk(ib - 1)
                 if pair == 1 and phases != "attn":
                     proj_chunk(n_ib - 1)

    nc.compile()
    return nc


def _shard_inputs(x, w_attn, b_attn, w_proj, b_proj, s=S):
    """Host-side sharding: build the per-core input maps."""
    x = np.asarray(x, dtype=np.float32)
    w_attn = np.asarray(w_attn, dtype=np.float32)
    b_attn = np.asarray(b_attn, dtype=np.float32)
    w_proj = np.asarray(w_proj, dtype=np.float32)
    b_proj = np.asarray(b_proj, dtype=np.float32)

    # causal mask tiles: msk[j, k, i] = 1.0 if i >= j + 128*k
    jj = np.arange(128)[:, None, None]
    kk = np.arange(4)[None, :, None]
    ii = np.arange(IB)[None, None, :]
    msk = (ii >= jj + 128 * kk).astype(np.float32)

    n_dt = D // 128
    in_maps = []
    for core in range(N_CORES):
        b, g = divmod(core, GROUP)
        hs = list(range(g * HLOC, (g + 1) * HLOC))
        # xt: [sh, 128, dt, IB] so each slab is one contiguous DMA
        xtT = np.ascontiguousarray(x[b].T)  # [D, S]
        xt = np.ascontiguousarray(
            xtT.reshape(n_dt, 128, S // IB, IB).transpose(2, 1, 0, 3)
        )
        qcols = np.concatenate(
            [w_attn[:, h * HD : (h + 1) * HD] for h in hs], axis=1
        )
        kcols = np.concatenate(
            [w_attn[:, D + h * HD : D + (h + 1) * HD] for h in hs], axis=1
        )
        # wqk col-groups t=0,1 (q pairs), 2,3 (k pairs): [128, dt, 128]
        wqk_t = {}
        for t in range(4):
            blk = (qcols if t < 2 else kcols)[:, (t % 2) * 128 : (t % 2) * 128 + 128]
            wqk_t[f"wqk{t}"] = np.ascontiguousarray(
                blk.reshape(n_dt, 128, 128).transpose(1, 0, 2)
            )
        # wv with a zero 65th column per head: [D, 260] -> [128, dt, 260]
        wv_full = np.zeros((D, 260), np.float32)
        for i, h in enumerate(hs):
            wv_full[:, i * 65 : i * 65 + 64] = w_attn[
                :, 2 * D + h * HD : 2 * D + (h + 1) * HD
            ]
        wv = np.ascontiguousarray(
            wv_full.reshape(n_dt, 128, 260).transpose(1, 0, 2)
        )
        bq = np.concatenate([b_attn[h * HD : (h + 1) * HD] for h in hs])
        bk = np.concatenate([b_attn[D + h * HD : D + (h + 1) * HD] for h in hs])
        bqk = np.concatenate([bq, bk]).reshape(4, 128).T.copy()  # [128, 4]
        # v bias + ones in the 65th column (softmax denominator source)
        bv_row = np.zeros(260, np.float32)
        for i, h in enumerate(hs):
            bv_row[i * 65 : i * 65 + 64] = b_attn[
                2 * D + h * HD : 2 * D + (h + 1) * HD
            ]
            bv_row[i * 65 + 64] = 1.0
        bv = np.broadcast_to(bv_row, (128, 260)).copy()
        # local 256 rows of w_proj as [128, pair, D]
        wrows = np.concatenate(
            [w_proj[h * HD : (h + 1) * HD, :] for h in hs], axis=0
        )  # [256, D]
        wpc = np.ascontiguousarray(
            wrows.reshape(NPAIR, 128, D).transpose(1, 0, 2)
        )
        in_maps.append(
            dict(
                xt=xt, wv=wv, bqk=bqk, bv=bv, wp=wpc, msk=msk,
                **wqk_t,
            )
        )
    return in_maps


def _unshard(results, b_proj):
    y = np.empty((B, S, D), np.float32)
    for b in range(B):
        acc = results[b * GROUP]["y"].astype(np.float32, copy=True)
        for g in range(1, GROUP):
            acc += results[b * GROUP + g]["y"]
        y[b] = acc + b_proj.astype(np.float32)
    return y


_NC_CACHE = {}


def kernel(x, w_attn, b_attn, w_proj, b_proj):
    from concourse.bass_utils import run_bass_kernel_spmd

    if S not in _NC_CACHE:
        _NC_CACHE[S] = _build_bass(S)
    nc = _NC_CACHE[S]
    in_maps = _shard_inputs(x, w_attn, b_attn, w_proj, b_proj)
    res = run_bass_kernel_spmd(nc, in_maps, list(range(N_CORES)))
    return _unshard(res.results, np.asarray(b_proj))
